# revision 16
# baseline (speedup 1.0000x reference)
"""Trainium2 Bass kernel for causal multi-head attention (v2).

Problem: B=2, T=2048, D=1024, H=16 heads of dim 64, causal softmax,
fp32 weights, no qkv bias, output projection with bias.

Sharding (8 cores): core c handles batch b = c//4 and head group
g = c%4 (4 heads = 256 of the 1024 qkv columns / out-proj rows).
Each core computes a partial output [T, D] (bf16) = ctx_heads @
Wo_slice; the host sums the 4 partials per batch in fp32 and adds bo.

Differences vs v1 (202us):
  - scores matmuls are K=64 row-tiled pairs: head h occupies SBUF
    partitions 64*(h%2).., so the two heads of a pair land on PE row
    groups 0/64 and execute CONCURRENTLY (tile_position auto-derived
    from base partitions) -> scores cost ~halves.
  - causal mask applied by accumulating a -1e5 bias block into the
    diagonal score PSUM via an identity matmul (PE), replacing the
    gpsimd probs multiply on the exp->AV critical path.
  - single fused exp per k-tile over both heads' scores [128, 2*qlen].
  - out-projection, V-projection and the mc=1 Q/K projections are
    emitted as PE "fillers" inside the attention loops, so the PE
    never drains during the ACT-bound attention phase and the output
    DMA is spread across the kernel instead of a 19us tail.
  - softmax normalization: lane-locked PSUM evictions (denominator
    rides row 64), reciprocal_approx_fast, and a K=1 f32r PE matmul
    broadcasts 1/den across partitions (no DRAM roundtrips); the
    whole chain is deferred into the next chunk's PE stream.
  - inputs arrive as xt[KO,P,T] (sync ring) and a fused wqkv[KO,P,768]
    (scalar ring) so DMA dispatch serialization halves; output is
    stored bf16.
"""

import os
import numpy as np
import ml_dtypes
from collections import deque

B, T, D = 2, 2048, 1024
H, HD = 16, 64
HC = 4          # heads per core
MC = HC * HD    # 256 qkv columns per core
P = 128
KO = D // P     # 8 contraction chunks for the projections
NT = T // P     # 16 token tiles
CW = 512        # attention q-chunk width
NCH = T // CW   # 4 q-chunks

_NC_CACHE = None


def _build_nc():
    import concourse.mybir as mybir
    import concourse.tile as tile
    from concourse import bacc
    from concourse.masks import make_identity

    dt = mybir.dt
    f32 = dt.float32
    f32r = dt.float32r
    bf16 = dt.bfloat16
    EXP = mybir.ActivationFunctionType.Exp

    nc = bacc.Bacc("TRN2", target_bir_lowering=False, debug=False, num_devices=8)

    # host pre-swizzled inputs (bf16)
    xtd = nc.dram_tensor("xtd", [KO, P, T], bf16, kind="ExternalInput").ap()
    wqkvd = nc.dram_tensor("wqkv", [KO, P, 3 * MC], bf16, kind="ExternalInput").ap()
    wod = nc.dram_tensor("wo", [2, P, D], bf16, kind="ExternalInput").ap()
    outd = nc.dram_tensor("out", [T, D], bf16, kind="ExternalOutput").ap()

    with tile.TileContext(nc) as tc:
        from contextlib import ExitStack

        with ExitStack() as ctx:
            pconst = ctx.enter_context(tc.tile_pool(name="pconst", bufs=1))
            pw = ctx.enter_context(tc.tile_pool(name="pw", bufs=1))
            pmain = ctx.enter_context(tc.tile_pool(name="pmain", bufs=1))
            psc = ctx.enter_context(tc.tile_pool(name="psc", bufs=2, space="PSUM"))
            pctx = ctx.enter_context(tc.tile_pool(name="pctx", bufs=1, space="PSUM"))
            pproj = ctx.enter_context(tc.tile_pool(name="pproj", bufs=2, space="PSUM"))
            pprob = ctx.enter_context(tc.tile_pool(name="pprob", bufs=4))
            pctxu = ctx.enter_context(tc.tile_pool(name="pctxu", bufs=2))
            pdeni = ctx.enter_context(tc.tile_pool(name="pdeni", bufs=2))
            pstage = ctx.enter_context(tc.tile_pool(name="pstage", bufs=2))
            pout = ctx.enter_context(tc.tile_pool(name="pout", bufs=2))
            prbc = ctx.enter_context(tc.tile_pool(name="prbc", bufs=2))
            pdram = ctx.enter_context(tc.tile_pool(name="pdram", bufs=2, space="DRAM"))

            # ---- persistent SBUF ----
            xt = pmain.tile([P, KO, T], bf16, tag="xt")          # X^T per-ko
            wqkv = pw.tile([P, KO, 3 * MC], bf16, tag="wqkv")
            wo_sb = pw.tile([P, 2, D], bf16, tag="wo")
            # per-head Q^T/K^T padded to K=128: head h occupies rows
            # 64*(h%2)..64*(h%2)+63 of slot h, complement rows are zeroed
            # (K<128 matmuls engage PE tiling modes that need drains between
            # mode switches — padding to K=128 keeps every matmul standard)
            qt = pmain.tile([P, HC, T], bf16, tag="qt")
            kt_sb = pmain.tile([P, HC, T], bf16, tag="kt")
            # V natural [k-token, per-(tt,h) 65-col block: 64 dims + ones]
            v_sb = pmain.tile([P, NT * HC * (HD + 1)], bf16, tag="v")
            ctxt = pmain.tile([P, 2, T], bf16, tag="ctxt")       # normalized ctx^T

            # ---- constants ----
            ones_f32 = pconst.tile([P, P], f32, tag="ones_f32")
            nc.vector.memset(ones_f32[:], 1.0)
            # zero the pad halves of qt/kt: mc0 slots on DVE (needed by the
            # first scores ~13us in), mc1 slots on the otherwise-idle gpsimd
            for s in range(2):
                lo = 64 * (1 - s % 2)
                nc.vector.memset(qt[lo : lo + 64, s, :], 0.0)
                nc.vector.memset(kt_sb[lo : lo + 64, s, :], 0.0)
            # ones column of each (tt, h) V block (denominator rides along AV)
            nc.vector.tensor_copy(
                v_sb[:].rearrange("p (t h c) -> p t h c", t=NT, h=HC)[:, :, :, HD],
                ones_f32[:, 0 : NT * HC].rearrange("p (t h) -> p t h", t=NT),
            )
            ident = pconst.tile([P, P], bf16, tag="ident")
            make_identity(nc, ident[:])
            # maskbias[k, q] = 0 if q >= k else -1e5  (bf16)
            maskbias = pconst.tile([P, P], bf16, tag="maskbias")
            nc.gpsimd.memset(maskbias[:], 0.0)
            nc.gpsimd.affine_select(
                out=maskbias[:],
                in_=maskbias[:],
                compare_op=mybir.AluOpType.is_ge,
                fill=-1e5,
                base=0,
                pattern=[[1, P]],
                channel_multiplier=-1,
            )
            for s in range(2, 4):
                lo = 64 * (1 - s % 2)
                nc.gpsimd.memset(qt[lo : lo + 64, s, :], 0.0)
                nc.gpsimd.memset(kt_sb[lo : lo + 64, s, :], 0.0)

            # ---- input DMAs: xt on the sync HWDGE ring, wqkv on the ACT
            # ring (dispatch serialization halves); per-ko granularity so
            # the first projection group starts after ~1/8 of the load ----
            for ko in range(KO):
                nc.sync.dma_start(xt[:, ko, :], xtd[ko])
                nc.scalar.dma_start(wqkv[:, ko, :], wqkvd[ko])
            nc.sync.dma_start(wo_sb[:, 0, :], wod[0])
            nc.sync.dma_start(wo_sb[:, 1, :], wod[1])

            # ================= emission machinery =================
            fillers = deque()
            normq = deque()

            def pop_fillers(k):
                for _ in range(min(k, len(fillers))):
                    fillers.popleft()()

            def qk_group(w, mc, th):
                # Q or K projection for head pair mc, 512-token chunk th
                def emit():
                    ps = pproj.tile([P, CW], f32, tag="pp", name="pp")
                    base = MC * w + P * mc
                    for ko in range(KO):
                        nc.tensor.matmul(
                            ps[:],
                            lhsT=wqkv[:, ko, base : base + P],
                            rhs=xt[:, ko, CW * th : CW * (th + 1)],
                            start=(ko == 0),
                            stop=(ko == KO - 1),
                        )
                    dst = qt if w == 0 else kt_sb
                    # even head -> slot 2mc rows 0-63 (DVE), odd head ->
                    # slot 2mc+1 rows 64-127 (ACT)
                    nc.vector.tensor_copy(
                        dst[0:64, 2 * mc, CW * th : CW * (th + 1)], ps[0:64]
                    )
                    nc.scalar.copy(
                        dst[64:P, 2 * mc + 1, CW * th : CW * (th + 1)], ps[64:P]
                    )
                return emit

            def v_group(tt):
                def emit():
                    ps = pproj.tile([P, CW], f32, tag="pp", name="pp")
                    for ko in range(KO):
                        nc.tensor.matmul(
                            ps[:, 0:MC],
                            lhsT=xt[:, ko, P * tt : P * (tt + 1)],
                            rhs=wqkv[:, ko, 2 * MC : 3 * MC],
                            start=(ko == 0),
                            stop=(ko == KO - 1),
                        )
                    nc.vector.tensor_copy(
                        v_sb[
                            :, tt * HC * (HD + 1) : (tt + 1) * HC * (HD + 1)
                        ].rearrange("p (h c) -> p h c", h=HC)[:, :, 0:HD],
                        ps[:, 0:MC].rearrange("p (h d) -> p h d", h=HC),
                    )
                return emit

            out_tiles = {}

            def out_group(tt, nn):
                # output projection for token tile tt, 512-col half nn
                def emit():
                    pp = pproj.tile([P, CW], f32, tag="pp", name="pp")
                    for mc in range(2):
                        nc.tensor.matmul(
                            pp[:],
                            lhsT=ctxt[:, mc, P * tt : P * (tt + 1)],
                            rhs=wo_sb[:, mc, CW * nn : CW * (nn + 1)],
                            start=(mc == 0),
                            stop=(mc == 1),
                        )
                    if nn == 0:
                        osb = pout.tile([P, D], bf16, tag="osb", name="osb")
                        out_tiles[tt] = osb
                        nc.vector.tensor_copy(osb[:, 0:CW], pp[:])
                    else:
                        osb = out_tiles.pop(tt)
                        nc.scalar.copy(osb[:, CW:D], pp[:])
                        nc.sync.dma_start(outd[P * tt : P * (tt + 1), :], osb[:])
                return emit

            def finish_norm(mc, qn, ctxu, scr):
                # deferred: DMA-broadcast 1/den across partitions (DRAM
                # bounce on the gpsimd SWDGE ring), normalize, odd-head move
                def emit():
                    qb = CW * qn
                    rbc = prbc.tile([P, 2, CW], f32, tag="rbc", name="rbc")
                    for hl in range(2):
                        nc.sync.dma_start(
                            rbc[:, hl, :], scr[hl : hl + 1, :].to_broadcast((P, CW))
                        )
                    nc.vector.tensor_mul(
                        ctxt[0:HD, mc, qb : qb + CW], ctxu[0:HD, 0, :], rbc[0:HD, 0, :]
                    )
                    stage = pstage.tile([HD, CW], bf16, tag="stage", name="stage")
                    nc.vector.tensor_mul(stage[:], ctxu[0:HD, 1, :], rbc[0:HD, 1, :])
                    nc.sync.dma_start(ctxt[HD:P, mc, qb : qb + CW], stage[:])
                return emit

            # ================= attention chunk =================
            def attn_chunk(mc, qn):
                qb = CW * qn
                nkt = 4 * qn + 4
                ctx2 = pctx.tile([P, 2, CW], f32, tag="ctx", name="ctx")
                probs_t = {}
                rels = {}

                def emit_sc(kti):
                    rel = max(0, P * kti - qb)
                    qlen = CW - rel
                    rels[kti] = (rel, qlen)
                    diag = P * kti >= qb
                    sc = psc.tile([P, 2, CW], f32, tag="sc", name="sc")
                    for hl in range(2):
                        nc.tensor.matmul(
                            sc[:, hl, 0:qlen],
                            lhsT=kt_sb[:, 2 * mc + hl, P * kti : P * (kti + 1)],
                            rhs=qt[:, 2 * mc + hl, qb + rel : qb + CW],
                            start=True,
                            stop=not diag,
                            skip_group_check=True,
                        )
                    if diag:
                        # additive causal mask, accumulated by the PE so the
                        # PSUM has_written semantics stay well-defined
                        for hl in range(2):
                            nc.tensor.matmul(
                                sc[:, hl, 0:P],
                                lhsT=ident[:],
                                rhs=maskbias[:],
                                start=False,
                                stop=True,
                                skip_group_check=True,
                            )
                    probs = pprob.tile([P, 2, CW], bf16, tag="probs", name="probs")
                    nc.scalar.activation(
                        probs[:, :, 0:qlen], sc[:, :, 0:qlen], EXP, scale=0.125
                    )
                    probs_t[kti] = probs

                def emit_av(kti):
                    rel, qlen = rels[kti]
                    probs = probs_t.pop(kti)
                    for hl in range(2):
                        vbase = (kti * HC + 2 * mc + hl) * (HD + 1)
                        nc.tensor.matmul(
                            ctx2[0 : HD + 1, hl, rel:CW],
                            lhsT=v_sb[:, vbase : vbase + HD + 1],
                            rhs=probs[:, hl, 0:qlen],
                            start=(kti == 0),
                            stop=(kti == nkt - 1),
                            skip_group_check=True,
                        )

                for kti in range(nkt):
                    if kti == 2 and normq:
                        normq.popleft()()
                    if (kti < 2 and mc == 0) or kti >= 3:
                        remaining = nkt - kti
                        k = (len(fillers) + remaining - 1) // remaining
                        pop_fillers(k)
                    emit_sc(kti)
                    if kti >= 1:
                        emit_av(kti - 1)
                emit_av(nkt - 1)

                # evict raw ctx^T (+ denominator rows) and start the recip;
                # the rest of the norm is deferred into the next chunk
                ctxu = pctxu.tile([HD + 1, 2, CW], f32, tag="ctxu", name="ctxu")
                nc.vector.tensor_copy(ctxu[:], ctx2[0 : HD + 1, :, :])
                deni = pdeni.tile([HD + 1, 2, CW], f32, tag="deni", name="deni")
                nc.vector.reciprocal(deni[HD : HD + 1, :, :], ctxu[HD : HD + 1, :, :])
                scr = pdram.tile([2, CW], f32, tag="scr", name="scr")
                nc.sync.dma_start(scr[:], deni[HD : HD + 1, :, :])
                normq.append(finish_norm(mc, qn, ctxu, scr))

            # ================= main schedule =================
            # prime the pipeline: Q/K for (mc0, th0) emitted directly
            qk_group(0, 0, 0)()
            qk_group(1, 0, 0)()

            for mc in range(2):
                for qn in range(NCH):
                    if mc == 0:
                        for j in range(4):
                            fillers.append(v_group(4 * qn + j))
                        if qn < NCH - 1:
                            fillers.append(qk_group(0, 0, qn + 1))
                            fillers.append(qk_group(1, 0, qn + 1))
                        fillers.append(qk_group(0, 1, qn))
                        fillers.append(qk_group(1, 1, qn))
                    else:
                        if qn >= 1:
                            for tt in range(4 * (qn - 1), 4 * qn):
                                fillers.append(out_group(tt, 0))
                                fillers.append(out_group(tt, 1))
                    attn_chunk(mc, qn)

            # tail: final norm + last 4 token tiles of the projection
            while normq:
                normq.popleft()()
            for tt in range(T // P - 4, T // P):
                fillers.append(out_group(tt, 0))
                fillers.append(out_group(tt, 1))
            while fillers:
                fillers.popleft()()

    nc.compile()
    return nc


def get_nc():
    global _NC_CACHE
    if _NC_CACHE is None:
        _NC_CACHE = _build_nc()
    return _NC_CACHE


def make_in_maps(x, Wq, Wk, Wv, Wo, bo):
    bf = ml_dtypes.bfloat16
    x = np.asarray(x, dtype=np.float32).astype(bf)
    Wq = np.asarray(Wq, dtype=np.float32).astype(bf)
    Wk = np.asarray(Wk, dtype=np.float32).astype(bf)
    Wv = np.asarray(Wv, dtype=np.float32).astype(bf)
    Wo = np.asarray(Wo, dtype=np.float32).astype(bf)
    in_maps = []
    for c in range(8):
        b, g = divmod(c, 4)
        sl = slice(MC * g, MC * (g + 1))
        xt_h = np.ascontiguousarray(x[b].T.reshape(KO, P, T))
        wqkv_h = np.ascontiguousarray(
            np.concatenate([Wq[:, sl], Wk[:, sl], Wv[:, sl]], axis=1).reshape(
                KO, P, 3 * MC
            )
        )
        wo_h = np.ascontiguousarray(Wo[sl, :].reshape(2, P, D))
        in_maps.append({"xtd": xt_h, "wqkv": wqkv_h, "wo": wo_h})
    return in_maps


def _install_profile_hook():
    """Register the axon NTFF profiling hook (the image's antenv lacks
    axon_hooks, so the boot-time registration degraded silently)."""
    import sys
    import types

    if "antenv.axon_hooks" not in sys.modules:
        m = types.ModuleType("antenv.axon_hooks")
        m._hook = None
        m.set_axon_ntff_profile_hook = lambda h: setattr(m, "_hook", h)
        m.get_axon_ntff_profile_hook = lambda: m._hook
        sys.modules["antenv.axon_hooks"] = m
        import antenv

        antenv.axon_hooks = m
    if "/root/.axon_site" not in sys.path:
        sys.path.append("/root/.axon_site")
    from trn_agent_boot.trn_boot import _ntff_profile_via_ctypes

    sys.modules["antenv.axon_hooks"].set_axon_ntff_profile_hook(
        _ntff_profile_via_ctypes("/opt/axon/libaxon_pjrt.so")
    )


def kernel_with_results(x, Wq, Wk, Wv, Wo, bo, trace=False):
    from concourse.bass_utils import run_bass_kernel_spmd

    if trace:
        _install_profile_hook()
    nc = get_nc()
    in_maps = make_in_maps(x, Wq, Wk, Wv, Wo, bo)
    res = run_bass_kernel_spmd(nc, in_maps, core_ids=list(range(8)), trace=trace)
    parts = [np.asarray(r["out"], dtype=np.float32) for r in res.results]
    bo32 = np.asarray(bo, dtype=np.float32).reshape(1, D)
    full = np.stack(
        [
            parts[0] + parts[1] + parts[2] + parts[3] + bo32,
            parts[4] + parts[5] + parts[6] + parts[7] + bo32,
        ]
    )
    return full, res


def kernel(x, Wq, Wk, Wv, Wo, bo):
    full, _ = kernel_with_results(
        x, Wq, Wk, Wv, Wo, bo, trace=bool(os.environ.get("KERNEL_TRACE"))
    )
    return full


# revision 19
# speedup vs baseline: 1.1927x; 1.1927x over previous
"""Trainium2 Bass kernel for causal multi-head attention (v2).

Problem: B=2, T=2048, D=1024, H=16 heads of dim 64, causal softmax,
fp32 weights, no qkv bias, output projection with bias.

Sharding (8 cores): core c handles batch b = c//4 and head group
g = c%4 (4 heads = 256 of the 1024 qkv columns / out-proj rows).
Each core computes a partial output [T, D] (bf16) = ctx_heads @
Wo_slice; the host sums the 4 partials per batch in fp32 and adds bo.

Differences vs v1 (202us):
  - scores matmuls are K=64 row-tiled pairs: head h occupies SBUF
    partitions 64*(h%2).., so the two heads of a pair land on PE row
    groups 0/64 and execute CONCURRENTLY (tile_position auto-derived
    from base partitions) -> scores cost ~halves.
  - causal mask applied by accumulating a -1e5 bias block into the
    diagonal score PSUM via an identity matmul (PE), replacing the
    gpsimd probs multiply on the exp->AV critical path.
  - single fused exp per k-tile over both heads' scores [128, 2*qlen].
  - out-projection, V-projection and the mc=1 Q/K projections are
    emitted as PE "fillers" inside the attention loops, so the PE
    never drains during the ACT-bound attention phase and the output
    DMA is spread across the kernel instead of a 19us tail.
  - softmax normalization: lane-locked PSUM evictions (denominator
    rides row 64), reciprocal_approx_fast, and a K=1 f32r PE matmul
    broadcasts 1/den across partitions (no DRAM roundtrips); the
    whole chain is deferred into the next chunk's PE stream.
  - inputs arrive as xt[KO,P,T] (sync ring) and a fused wqkv[KO,P,768]
    (scalar ring) so DMA dispatch serialization halves; output is
    stored bf16.
"""

import os
import numpy as np
import ml_dtypes
from collections import deque

B, T, D = 2, 2048, 1024
H, HD = 16, 64
HC = 4          # heads per core
MC = HC * HD    # 256 qkv columns per core
P = 128
KO = D // P     # 8 contraction chunks for the projections
NT = T // P     # 16 token tiles
CW = 512        # attention q-chunk width
NCH = T // CW   # 4 q-chunks

_NC_CACHE = None


def _build_nc():
    import concourse.mybir as mybir
    import concourse.tile as tile
    from concourse import bacc
    from concourse.masks import make_identity

    dt = mybir.dt
    f32 = dt.float32
    f32r = dt.float32r
    bf16 = dt.bfloat16
    EXP = mybir.ActivationFunctionType.Exp

    nc = bacc.Bacc("TRN2", target_bir_lowering=False, debug=False, num_devices=8)

    # host pre-swizzled inputs (bf16)
    xtd = nc.dram_tensor("xtd", [KO, P, T], bf16, kind="ExternalInput").ap()
    wqkvd = nc.dram_tensor("wqkv", [KO, P, 3 * MC], bf16, kind="ExternalInput").ap()
    wod = nc.dram_tensor("wo", [2, P, D], bf16, kind="ExternalInput").ap()
    outd = nc.dram_tensor("out", [T, D], bf16, kind="ExternalOutput").ap()

    with tile.TileContext(nc) as tc:
        from contextlib import ExitStack

        with ExitStack() as ctx:
            pconst = ctx.enter_context(tc.tile_pool(name="pconst", bufs=1))
            pw = ctx.enter_context(tc.tile_pool(name="pw", bufs=1))
            pmain = ctx.enter_context(tc.tile_pool(name="pmain", bufs=1))
            psc = ctx.enter_context(tc.tile_pool(name="psc", bufs=2, space="PSUM"))
            pctx = ctx.enter_context(tc.tile_pool(name="pctx", bufs=1, space="PSUM"))
            pproj = ctx.enter_context(tc.tile_pool(name="pproj", bufs=2, space="PSUM"))
            pprob = ctx.enter_context(tc.tile_pool(name="pprob", bufs=4))
            pctxu = ctx.enter_context(tc.tile_pool(name="pctxu", bufs=2))
            pdeni = ctx.enter_context(tc.tile_pool(name="pdeni", bufs=2))
            pstage = ctx.enter_context(tc.tile_pool(name="pstage", bufs=2))
            pout = ctx.enter_context(tc.tile_pool(name="pout", bufs=2))
            prbc = ctx.enter_context(tc.tile_pool(name="prbc", bufs=2))
            pdram = ctx.enter_context(tc.tile_pool(name="pdram", bufs=2, space="DRAM"))

            # ---- persistent SBUF ----
            xt = pmain.tile([P, KO, T], bf16, tag="xt")          # X^T per-ko
            wqkv = pw.tile([P, KO, 3 * MC], bf16, tag="wqkv")
            wo_sb = pw.tile([P, 2, D], bf16, tag="wo")
            # per-head Q^T/K^T padded to K=128: head h occupies rows
            # 64*(h%2)..64*(h%2)+63 of slot h, complement rows are zeroed
            # (K<128 matmuls engage PE tiling modes that need drains between
            # mode switches — padding to K=128 keeps every matmul standard)
            qt = pmain.tile([P, HC, T], bf16, tag="qt")
            kt_sb = pmain.tile([P, HC, T], bf16, tag="kt")
            # V natural [k-token, per-(tt,h) 65-col block: 64 dims + ones]
            v_sb = pmain.tile([P, NT * HC * (HD + 1)], bf16, tag="v")
            ctxt = pmain.tile([P, 2, T], bf16, tag="ctxt")       # normalized ctx^T

            # ---- constants ----
            ones_f32 = pconst.tile([P, P], f32, tag="ones_f32")
            nc.vector.memset(ones_f32[:], 1.0)
            # zero the pad halves of qt/kt: mc0 slots on DVE (needed by the
            # first scores ~13us in), mc1 slots on the otherwise-idle gpsimd
            for s in range(2):
                lo = 64 * (1 - s % 2)
                nc.vector.memset(qt[lo : lo + 64, s, :], 0.0)
                nc.vector.memset(kt_sb[lo : lo + 64, s, :], 0.0)
            # ones column of each (tt, h) V block (denominator rides along AV)
            nc.vector.tensor_copy(
                v_sb[:].rearrange("p (t h c) -> p t h c", t=NT, h=HC)[:, :, :, HD],
                ones_f32[:, 0 : NT * HC].rearrange("p (t h) -> p t h", t=NT),
            )
            ident = pconst.tile([P, P], bf16, tag="ident")
            make_identity(nc, ident[:])
            # maskbias[k, q] = 0 if q >= k else -1e5  (bf16)
            maskbias = pconst.tile([P, P], bf16, tag="maskbias")
            nc.gpsimd.memset(maskbias[:], 0.0)
            nc.gpsimd.affine_select(
                out=maskbias[:],
                in_=maskbias[:],
                compare_op=mybir.AluOpType.is_ge,
                fill=-1e5,
                base=0,
                pattern=[[1, P]],
                channel_multiplier=-1,
            )
            for s in range(2, 4):
                lo = 64 * (1 - s % 2)
                nc.gpsimd.memset(qt[lo : lo + 64, s, :], 0.0)
                nc.gpsimd.memset(kt_sb[lo : lo + 64, s, :], 0.0)

            # ---- input DMAs: xt on the sync HWDGE ring, wqkv on the ACT
            # ring (dispatch serialization halves); per-ko granularity so
            # the first projection group starts after ~1/8 of the load ----
            for ko in range(KO):
                nc.sync.dma_start(xt[:, ko, :], xtd[ko])
                nc.scalar.dma_start(wqkv[:, ko, :], wqkvd[ko])
            nc.sync.dma_start(wo_sb[:, 0, :], wod[0])
            nc.sync.dma_start(wo_sb[:, 1, :], wod[1])

            # ================= emission machinery =================
            fillers = deque()
            normq = deque()

            def pop_fillers(k):
                for _ in range(min(k, len(fillers))):
                    fillers.popleft()()

            def qk_group(w, mc, th):
                # Q or K projection for head pair mc, 512-token chunk th
                def emit():
                    ps = pproj.tile([P, CW], f32, tag="pp", name="pp")
                    base = MC * w + P * mc
                    for ko in range(KO):
                        nc.tensor.matmul(
                            ps[:],
                            lhsT=wqkv[:, ko, base : base + P],
                            rhs=xt[:, ko, CW * th : CW * (th + 1)],
                            start=(ko == 0),
                            stop=(ko == KO - 1),
                        )
                    dst = qt if w == 0 else kt_sb
                    # even head -> slot 2mc rows 0-63 (DVE), odd head ->
                    # slot 2mc+1 rows 64-127 (ACT)
                    nc.vector.tensor_copy(
                        dst[0:64, 2 * mc, CW * th : CW * (th + 1)], ps[0:64]
                    )
                    nc.scalar.copy(
                        dst[64:P, 2 * mc + 1, CW * th : CW * (th + 1)], ps[64:P]
                    )
                return emit

            def v_group(tt):
                def emit():
                    ps = pproj.tile([P, CW], f32, tag="pp", name="pp")
                    for ko in range(KO):
                        nc.tensor.matmul(
                            ps[:, 0:MC],
                            lhsT=xt[:, ko, P * tt : P * (tt + 1)],
                            rhs=wqkv[:, ko, 2 * MC : 3 * MC],
                            start=(ko == 0),
                            stop=(ko == KO - 1),
                        )
                    nc.vector.tensor_copy(
                        v_sb[
                            :, tt * HC * (HD + 1) : (tt + 1) * HC * (HD + 1)
                        ].rearrange("p (h c) -> p h c", h=HC)[:, :, 0:HD],
                        ps[:, 0:MC].rearrange("p (h d) -> p h d", h=HC),
                    )
                return emit

            out_tiles = {}

            def out_group(tt, nn):
                # output projection for token tile tt, 512-col half nn
                def emit():
                    pp = pproj.tile([P, CW], f32, tag="pp", name="pp")
                    for mc in range(2):
                        nc.tensor.matmul(
                            pp[:],
                            lhsT=ctxt[:, mc, P * tt : P * (tt + 1)],
                            rhs=wo_sb[:, mc, CW * nn : CW * (nn + 1)],
                            start=(mc == 0),
                            stop=(mc == 1),
                        )
                    if nn == 0:
                        osb = pout.tile([P, D], bf16, tag="osb", name="osb")
                        out_tiles[tt] = osb
                        nc.vector.tensor_copy(osb[:, 0:CW], pp[:])
                    else:
                        osb = out_tiles.pop(tt)
                        nc.scalar.copy(osb[:, CW:D], pp[:])
                        nc.sync.dma_start(outd[P * tt : P * (tt + 1), :], osb[:])
                return emit

            def finish_norm(mc, qn, ctxu, den64):
                # deferred: recip on 64 lanes, gather to DRAM, DMA-broadcast
                # 1/den across partitions, normalize, odd-head move
                def emit():
                    qb = CW * qn
                    deni = pdeni.tile([HD, 16], f32, tag="deni", name="deni")
                    nc.vector.reciprocal(deni[:], den64[:])
                    scr = pdram.tile([2, CW], f32, tag="scr", name="scr")
                    nc.sync.dma_start(
                        scr[:].rearrange("a (p b) -> (a p) b", p=HD // 2), deni[:]
                    )
                    rbc = prbc.tile([P, 2, CW], f32, tag="rbc", name="rbc")
                    for hl in range(2):
                        nc.sync.dma_start(
                            rbc[:, hl, :], scr[hl : hl + 1, :].to_broadcast((P, CW))
                        )
                    nc.vector.tensor_mul(
                        ctxt[0:HD, mc, qb : qb + CW], ctxu[0:HD, 0, :], rbc[0:HD, 0, :]
                    )
                    stage = pstage.tile([HD, CW], bf16, tag="stage", name="stage")
                    nc.vector.tensor_mul(stage[:], ctxu[0:HD, 1, :], rbc[0:HD, 1, :])
                    nc.sync.dma_start(ctxt[HD:P, mc, qb : qb + CW], stage[:])
                return emit

            # ================= attention chunk =================
            def attn_chunk(mc, qn):
                qb = CW * qn
                nkt = 4 * qn + 4
                ctx2 = pctx.tile([P, 2, CW], f32, tag="ctx", name="ctx")
                probs_t = {}
                rels = {}

                def emit_sc(kti):
                    rel = max(0, P * kti - qb)
                    qlen = CW - rel
                    rels[kti] = (rel, qlen)
                    diag = P * kti >= qb
                    sc = psc.tile([P, 2, CW], f32, tag="sc", name="sc")
                    for hl in range(2):
                        nc.tensor.matmul(
                            sc[:, hl, 0:qlen],
                            lhsT=kt_sb[:, 2 * mc + hl, P * kti : P * (kti + 1)],
                            rhs=qt[:, 2 * mc + hl, qb + rel : qb + CW],
                            start=True,
                            stop=not diag,
                            skip_group_check=True,
                        )
                    if diag:
                        # additive causal mask, accumulated by the PE so the
                        # PSUM has_written semantics stay well-defined
                        for hl in range(2):
                            nc.tensor.matmul(
                                sc[:, hl, 0:P],
                                lhsT=ident[:],
                                rhs=maskbias[:],
                                start=False,
                                stop=True,
                                skip_group_check=True,
                            )
                    probs = pprob.tile([P, 2, CW], bf16, tag="probs", name="probs")
                    nc.scalar.activation(
                        probs[:, :, 0:qlen], sc[:, :, 0:qlen], EXP, scale=0.125
                    )
                    probs_t[kti] = probs

                def emit_av(kti):
                    rel, qlen = rels[kti]
                    probs = probs_t.pop(kti)
                    for hl in range(2):
                        vbase = (kti * HC + 2 * mc + hl) * (HD + 1)
                        nc.tensor.matmul(
                            ctx2[0 : HD + 1, hl, rel:CW],
                            lhsT=v_sb[:, vbase : vbase + HD + 1],
                            rhs=probs[:, hl, 0:qlen],
                            start=(kti == 0),
                            stop=(kti == nkt - 1),
                            skip_group_check=True,
                        )

                for kti in range(nkt):
                    if kti == 2 and normq:
                        normq.popleft()()
                    if (kti < 2 and mc == 0) or kti >= 3:
                        remaining = nkt - kti
                        k = (len(fillers) + remaining - 1) // remaining
                        pop_fillers(k)
                    emit_sc(kti)
                    if kti >= 1:
                        emit_av(kti - 1)
                emit_av(nkt - 1)

                # evict raw ctx^T (+ denominator rows) and start the recip;
                # the rest of the norm is deferred into the next chunk
                ctxu = pctxu.tile([HD + 1, 2, CW], f32, tag="ctxu", name="ctxu")
                nc.vector.tensor_copy(ctxu[:], ctx2[0 : HD + 1, :, :])
                # spread the 1024 denominators over 64 lanes so the exact
                # reciprocal costs ~16 columns instead of 1024
                den64 = pdeni.tile([HD, 16], f32, tag="den64", name="den64")
                nc.sync.dma_start(den64[:], ctxu[HD : HD + 1, :, :])
                normq.append(finish_norm(mc, qn, ctxu, den64))

            # ================= main schedule =================
            # prime the pipeline: Q/K for (mc0, th0) emitted directly
            qk_group(0, 0, 0)()
            qk_group(1, 0, 0)()

            for mc in range(2):
                for qn in range(NCH):
                    if mc == 0:
                        for j in range(4):
                            fillers.append(v_group(4 * qn + j))
                        if qn < NCH - 1:
                            fillers.append(qk_group(0, 0, qn + 1))
                            fillers.append(qk_group(1, 0, qn + 1))
                        fillers.append(qk_group(0, 1, qn))
                        fillers.append(qk_group(1, 1, qn))
                    else:
                        if qn >= 1:
                            for tt in range(4 * (qn - 1), 4 * qn):
                                fillers.append(out_group(tt, 0))
                                fillers.append(out_group(tt, 1))
                    attn_chunk(mc, qn)

            # tail: final norm + last 4 token tiles of the projection
            while normq:
                normq.popleft()()
            for tt in range(T // P - 4, T // P):
                fillers.append(out_group(tt, 0))
                fillers.append(out_group(tt, 1))
            while fillers:
                fillers.popleft()()

    nc.compile()
    return nc


def get_nc():
    global _NC_CACHE
    if _NC_CACHE is None:
        _NC_CACHE = _build_nc()
    return _NC_CACHE


def make_in_maps(x, Wq, Wk, Wv, Wo, bo):
    bf = ml_dtypes.bfloat16
    x = np.asarray(x, dtype=np.float32).astype(bf)
    Wq = np.asarray(Wq, dtype=np.float32).astype(bf)
    Wk = np.asarray(Wk, dtype=np.float32).astype(bf)
    Wv = np.asarray(Wv, dtype=np.float32).astype(bf)
    Wo = np.asarray(Wo, dtype=np.float32).astype(bf)
    in_maps = []
    for c in range(8):
        b, g = divmod(c, 4)
        sl = slice(MC * g, MC * (g + 1))
        xt_h = np.ascontiguousarray(x[b].T.reshape(KO, P, T))
        wqkv_h = np.ascontiguousarray(
            np.concatenate([Wq[:, sl], Wk[:, sl], Wv[:, sl]], axis=1).reshape(
                KO, P, 3 * MC
            )
        )
        wo_h = np.ascontiguousarray(Wo[sl, :].reshape(2, P, D))
        in_maps.append({"xtd": xt_h, "wqkv": wqkv_h, "wo": wo_h})
    return in_maps


def _install_profile_hook():
    """Register the axon NTFF profiling hook (the image's antenv lacks
    axon_hooks, so the boot-time registration degraded silently)."""
    import sys
    import types

    if "antenv.axon_hooks" not in sys.modules:
        m = types.ModuleType("antenv.axon_hooks")
        m._hook = None
        m.set_axon_ntff_profile_hook = lambda h: setattr(m, "_hook", h)
        m.get_axon_ntff_profile_hook = lambda: m._hook
        sys.modules["antenv.axon_hooks"] = m
        import antenv

        antenv.axon_hooks = m
    if "/root/.axon_site" not in sys.path:
        sys.path.append("/root/.axon_site")
    from trn_agent_boot.trn_boot import _ntff_profile_via_ctypes

    sys.modules["antenv.axon_hooks"].set_axon_ntff_profile_hook(
        _ntff_profile_via_ctypes("/opt/axon/libaxon_pjrt.so")
    )


def kernel_with_results(x, Wq, Wk, Wv, Wo, bo, trace=False):
    from concourse.bass_utils import run_bass_kernel_spmd

    if trace:
        _install_profile_hook()
    nc = get_nc()
    in_maps = make_in_maps(x, Wq, Wk, Wv, Wo, bo)
    res = run_bass_kernel_spmd(nc, in_maps, core_ids=list(range(8)), trace=trace)
    parts = [np.asarray(r["out"], dtype=np.float32) for r in res.results]
    bo32 = np.asarray(bo, dtype=np.float32).reshape(1, D)
    full = np.stack(
        [
            parts[0] + parts[1] + parts[2] + parts[3] + bo32,
            parts[4] + parts[5] + parts[6] + parts[7] + bo32,
        ]
    )
    return full, res


def kernel(x, Wq, Wk, Wv, Wo, bo):
    full, _ = kernel_with_results(
        x, Wq, Wk, Wv, Wo, bo, trace=bool(os.environ.get("KERNEL_TRACE"))
    )
    return full
